# revision 20
# baseline (speedup 1.0000x reference)
"""Causal self-attention (B=4, S=2048, D=1024, H=16, hd=64) on 8 TRN2 NeuronCores.

Sharding: batch 4-way x head-group 2-way. Core c = 2*b + g handles batch b and
heads [8g, 8g+8). Each core computes the QKV projection for its heads, causal
flash-style attention, and a partial output projection; the host sums the two
head-group partials per batch.

Per-core kernel layout choices:
  - q^T / k^T are produced in [hd, S] layout (head-dim on partitions) directly
    from the projection, V in [S, hd] layout via a second projection pass with
    x^T tiles as the stationary operand.
  - Attention computes S^T = K.Q^T per (128-kv-chunk x 512-q-superblock), so
    the exp() activation's PSUM->SBUF pass lands P^T directly in the layout the
    P^T.V matmul wants. A ones-column appended to V yields the softmax
    denominators from the same matmul (row 64 of the accumulator).
  - No running-max subtraction: scores are bounded (|s|/8 < ~30) so exp stays
    finite in fp32; masked positions get -1e10 before the 0.125 scale.
"""

import sys

for _p in ("/opt/trn_rl_repo",):
    if _p not in sys.path:
        sys.path.insert(0, _p)

from contextlib import ExitStack

import numpy as np

import concourse.bass as bass
import concourse.mybir as mybir
import concourse.tile as tile
from concourse import bacc
from concourse.bass_utils import run_bass_kernel_spmd

F32 = mybir.dt.float32
BF16 = mybir.dt.bfloat16
P = 128
B, S, D = 4, 2048, 1024
HD = 64          # head dim
NH = 8           # heads per core
KO = D // P      # 8 contraction chunks for the projections
QSB = 512        # q superblock (matmul free dim)
N_SB = S // QSB  # 4
N_SC = S // P    # 16 kv chunks
PSTRIPE = 512    # s-stripe for the projection phase
NEG = -1.0e10
SCALE = 0.125    # 1/sqrt(64)


def _attention_kernel(tc, out, xT, w_qk, w_v, w_out):
    nc = tc.nc
    with ExitStack() as ctx:
        const_pool = ctx.enter_context(tc.tile_pool(name="const", bufs=1))
        qkT_pool = ctx.enter_context(tc.tile_pool(name="qkT", bufs=1))
        v_pool = ctx.enter_context(tc.tile_pool(name="vsb", bufs=1))
        wqk_pool = ctx.enter_context(tc.tile_pool(name="wqk", bufs=1))
        wv_pool = ctx.enter_context(tc.tile_pool(name="wv", bufs=1))
        wout_pool = ctx.enter_context(tc.tile_pool(name="wout", bufs=1))
        xt_pool = ctx.enter_context(tc.tile_pool(name="xt", bufs=2))
        pt_pool = ctx.enter_context(tc.tile_pool(name="pt", bufs=6))
        y_pool = ctx.enter_context(tc.tile_pool(name="ysb", bufs=2))
        r_pool = ctx.enter_context(tc.tile_pool(name="recip", bufs=6))
        o_pool = ctx.enter_context(tc.tile_pool(name="osb", bufs=2))
        # one shared PSUM pool of 2-bank tiles (plus the two y accumulators):
        # 3*2 + 2*1 = 8 banks. Projection, attention scores, and the out-proj
        # accumulators all rotate through ps_s2 so projection stripes can
        # execute concurrently with attention on earlier stripes.
        ps_s2 = ctx.enter_context(tc.tile_pool(name="ps_s2", bufs=3, space="PSUM"))
        ps_y = ctx.enter_context(tc.tile_pool(name="ps_y", bufs=2, space="PSUM"))

        # 128x128 triangle for the diagonal block (transposed layout):
        # tri[i, j] = 1 if j >= i else 0
        tri = const_pool.tile([P, P], BF16, tag="tri")
        nc.gpsimd.memset(tri[:], 1.0)
        nc.gpsimd.affine_select(
            out=tri[:],
            in_=tri[:],
            compare_op=mybir.AluOpType.is_ge,
            fill=0.0,
            base=0,
            channel_multiplier=-1,
            pattern=[[1, P]],
        )

        # q^T/k^T store: row-chunk rc<4 holds q rows, rc>=4 holds k rows.
        # Head h lives at partitions 64*(h%2)..+64 of row-chunk h//2 (+4 for k).
        qkT = qkT_pool.tile([P, 8, S], BF16)
        # V store: [s-partition, kv-chunk, head, hd+1]; last col is ones for the
        # softmax denominator.
        v_sb = v_pool.tile([P, N_SC, NH, HD + 1], BF16)
        nc.gpsimd.memset(v_sb[:, :, :, HD], 1.0)

        # stripe-0 x chunks interleave with the weight chunks so the first
        # projection matmul starts after ~2 chunks instead of the full 5 MB
        wqk_sb = wqk_pool.tile([P, KO, 2 * 512], BF16)
        xt0 = xt_pool.tile([P, KO, PSTRIPE], BF16, tag="xt", name="xt_first")
        for ko in range(KO):
            nc.sync.dma_start(
                wqk_sb[:, ko, :],
                w_qk[ko * P:(ko + 1) * P, :],
            )
            nc.sync.dma_start(xt0[:, ko, :], xT[ko * P:(ko + 1) * P, 0:PSTRIPE])
        wv_sb = wv_pool.tile([P, KO, 512], BF16)
        nc.sync.dma_start(wv_sb[:], w_v.rearrange("(ko ki) n -> ki ko n", ki=P))
        wout_sb = wout_pool.tile([P, 4, D], BF16)
        nc.sync.dma_start(wout_sb[:], w_out.rearrange("(co ci) n -> ci co n", ci=P))

        def proj_stripe(st, pieces=None, xt_pre=None):
            if xt_pre is not None:
                xt = xt_pre
            else:
                xt = xt_pool.tile([P, KO, PSTRIPE], BF16, tag="xt", name=f"xt{st}")
                for ko in range(KO):
                    nc.sync.dma_start(
                        xt[:, ko, :],
                        xT[ko * P:(ko + 1) * P, st * PSTRIPE:(st + 1) * PSTRIPE],
                    )
            # q^T/k^T rows, two row-chunks per 2-bank psum tile
            def qk_piece(rcp):
                ps = ps_s2.tile([P, 2, PSTRIPE], F32, tag="ps_s2", name=f"pqk{rcp}")
                for half in range(2):
                    rc = 2 * rcp + half
                    for ko in range(KO):
                        nc.tensor.matmul(
                            ps[:, half, :],
                            lhsT=wqk_sb[:, ko, rc * P:(rc + 1) * P],
                            rhs=xt[:, ko, :],
                            start=(ko == 0),
                            stop=(ko == KO - 1),
                        )
                nc.vector.tensor_copy(
                    qkT[:, 2 * rcp:2 * rcp + 2, st * PSTRIPE:(st + 1) * PSTRIPE],
                    ps[:],
                )
            # V rows, two s-chunks per psum tile
            def v_piece(subp):
                ps = ps_s2.tile([P, 2, NH * HD], F32, tag="ps_s2", name=f"pv{subp}")
                for half in range(2):
                    sub = 2 * subp + half
                    for ko in range(KO):
                        nc.tensor.matmul(
                            ps[:, half, :],
                            lhsT=xt[:, ko, sub * P:(sub + 1) * P],
                            rhs=wv_sb[:, ko, :],
                            start=(ko == 0),
                            stop=(ko == KO - 1),
                        )
                sc0 = st * (PSTRIPE // P) + 2 * subp
                nc.vector.tensor_copy(
                    v_sb[:, sc0:sc0 + 2, :, 0:HD],
                    ps.rearrange("p t (h e) -> p t h e", h=NH),
                )

            todo = [lambda r=r: qk_piece(r) for r in range(4)]
            todo += [lambda s_=s_: v_piece(s_) for s_ in range(PSTRIPE // P // 2)]
            if pieces is None:
                for fn in todo:
                    fn()
            else:
                pieces.extend(todo)

        def attn_sb(sb, pieces=()):
            pieces = list(pieces)
            n_slots = (NH // 2) * (2 * (sb + 1))
            stride = max(1, n_slots // max(1, len(pieces)))
            slot = [0]
            ySb = y_pool.tile([P, 4, QSB], BF16, tag="ysb", name=f"ysb{sb}")
            # head pairs: h0/h1 share a row-chunk at base partitions 0/64, so
            # their QK matmuls land on disjoint PE row-groups (issued
            # adjacently for array-level concurrency) and the interleave keeps
            # the PE fed across the exp() dependency.
            for hp in range(NH // 2):
                heads = (2 * hp, 2 * hp + 1)
                rc_k = 4 + hp
                y_pss = [
                    ps_y.tile([P, QSB], F32, tag="ps_y", name=f"yps{i}")
                    for i in range(2)
                ]
                nch = 4 * (sb + 1)
                for g in range(nch // 2):
                    c0 = 2 * g
                    # causal trim: chunk c only attends q >= c*128, i.e.
                    # columns qoff.. of this superblock; the diagonal 128x128
                    # block gets a triangular mask on P^T instead of a full
                    # additive mask pass.
                    qoffs = [P * max(0, c0 + i - 4 * sb) for i in range(2)]
                    s2s = [
                        ps_s2.tile([P, 2, QSB], F32, tag="ps_s2", name=f"s2_{i}")
                        for i in range(2)
                    ]
                    for i in range(2):
                        qo = qoffs[i]
                        for (h, s2) in zip(heads, s2s):
                            bp = (h % 2) * HD
                            c = c0 + i
                            nc.tensor.matmul(
                                s2[:, i, qo:],
                                lhsT=qkT[bp:bp + HD, rc_k, c * P:(c + 1) * P],
                                rhs=qkT[bp:bp + HD, hp, sb * QSB + qo:(sb + 1) * QSB],
                                start=True,
                                stop=True,
                            )
                    for (h, s2, y_ps) in zip(heads, s2s, y_pss):
                        bp = (h % 2) * HD
                        pt = pt_pool.tile([P, 2, QSB], BF16, tag="pt")
                        ptb = pt[:]
                        if qoffs[0] == qoffs[1]:
                            nc.scalar.activation(
                                pt[:, :, qoffs[0]:], s2[:, :, qoffs[0]:],
                                mybir.ActivationFunctionType.Exp,
                                scale=SCALE,
                            )
                        else:
                            for i in range(2):
                                nc.scalar.activation(
                                    pt[:, i, qoffs[i]:], s2[:, i, qoffs[i]:],
                                    mybir.ActivationFunctionType.Exp,
                                    scale=SCALE,
                                )
                        for i in range(2):
                            c = c0 + i
                            qo = qoffs[i]
                            if c >= 4 * sb:
                                # triangle at the causal diagonal block
                                nc.vector.tensor_tensor(
                                    ptb[:, i, qo:qo + P],
                                    ptb[:, i, qo:qo + P],
                                    tri[:],
                                    mybir.AluOpType.mult,
                                )
                            nc.tensor.matmul(
                                y_ps[0:HD + 1, qo:],
                                lhsT=v_sb[:, c, h, :],
                                rhs=ptb[:, i, qo:],
                                start=(c == 0),
                                stop=(c == nch - 1),
                            )
                    slot[0] += 1
                    if pieces and slot[0] % stride == 0:
                        pieces.pop(0)()
                if hp == NH // 2 - 1:
                    # flush any remainder before the out-projection
                    while pieces:
                        pieces.pop(0)()
                for h, y_ps in zip(heads, y_pss):
                    bp = (h % 2) * HD
                    # one copy releases the PSUM accumulator immediately (the
                    # next head-pair's AVs need the bank); the normalize chain
                    # then runs off the SBUF copy.
                    yc = r_pool.tile([HD + 1, QSB], F32, tag="yc")
                    nc.vector.tensor_copy(yc[:], y_ps[0:HD + 1, :])
                    # stage sums at partition 0: reciprocal_approx_fast
                    # (custom DVE op) reads garbage from nonzero base
                    # partitions on HW
                    ssum = r_pool.tile([1, QSB], F32, tag="ssum")
                    nc.vector.tensor_copy(ssum[:], yc[HD:HD + 1, :])
                    r = r_pool.tile([1, QSB], F32, tag="r")
                    nc.vector.reciprocal_approx_fast(r[:], ssum[:])
                    rbc = r_pool.tile([HD, QSB], F32, tag="rbc")
                    nc.gpsimd.partition_broadcast(rbc[:], r[:])
                    nc.vector.tensor_tensor(
                        ySb[bp:bp + HD, hp, :], yc[0:HD, :], rbc[:],
                        mybir.AluOpType.mult,
                    )
            # output projection for this superblock's s-range
            for sub in range(QSB // P):
                o_t = o_pool.tile([P, 2, 512], F32, tag="osb")
                o_ps = ps_s2.tile([P, 2, 512], F32, tag="ps_s2", name="ops")
                for nt in range(2):
                    for cc in range(4):
                        nc.tensor.matmul(
                            o_ps[:, nt, :],
                            lhsT=ySb[:, cc, sub * P:(sub + 1) * P],
                            rhs=wout_sb[:, cc, nt * 512:(nt + 1) * 512],
                            start=(cc == 0),
                            stop=(cc == 3),
                        )
                nc.vector.tensor_copy(o_t[:], o_ps[:])
                row = (sb * (QSB // P) + sub) * P
                nc.sync.dma_start(out[row:row + P, :], o_t.rearrange("p a b -> p (a b)"))

        # dovetail: attention on superblock sb only needs projection stripes
        # <= sb, so stripe sb+1's pieces are interleaved between attention
        # head-pairs of superblock sb (keeps ACT fed with exps while the PE
        # chews projection matmuls).
        proj_stripe(0, xt_pre=xt0)
        for sb in range(N_SB):
            pieces = []
            if sb + 1 < N_SB:
                proj_stripe(sb + 1, pieces)
            attn_sb(sb, pieces)


_NC_CACHE = None


def _build_program():
    global _NC_CACHE
    if _NC_CACHE is not None:
        return _NC_CACHE
    nc = bacc.Bacc("TRN2", target_bir_lowering=False, debug=False)
    xT = nc.dram_tensor("xT", [D, S], BF16, kind="ExternalInput").ap()
    w_qk = nc.dram_tensor("w_qk", [D, 1024], BF16, kind="ExternalInput").ap()
    w_v = nc.dram_tensor("w_v", [D, 512], BF16, kind="ExternalInput").ap()
    w_out = nc.dram_tensor("w_out", [512, D], BF16, kind="ExternalInput").ap()
    out = nc.dram_tensor("out", [S, D], F32, kind="ExternalOutput").ap()
    with tile.TileContext(nc) as tc:
        _attention_kernel(tc, out, xT, w_qk, w_v, w_out)
    nc.compile()
    _NC_CACHE = nc
    return nc


def make_in_maps(x, W_qkv, W_out):
    import ml_dtypes

    bf16 = ml_dtypes.bfloat16
    x = np.ascontiguousarray(np.asarray(x, dtype=np.float32))
    W_qkv = np.asarray(W_qkv, dtype=np.float32)
    W_out = np.asarray(W_out, dtype=np.float32)
    in_maps = []
    for c in range(8):
        b, g = divmod(c, 2)
        lo = 512 * g
        cols = np.arange(lo, lo + 512)
        in_maps.append({
            "xT": np.ascontiguousarray(x[b].T).astype(bf16),
            "w_qk": np.ascontiguousarray(
                np.concatenate([W_qkv[:, cols], W_qkv[:, D + cols]], axis=1)
            ).astype(bf16),
            "w_v": np.ascontiguousarray(W_qkv[:, 2 * D + cols]).astype(bf16),
            "w_out": np.ascontiguousarray(W_out[cols, :]).astype(bf16),
        })
    return in_maps


def combine_outputs(results):
    # results: list of 8 dicts with "out" [S, D]; core c = 2*b + g
    return np.stack(
        [results[2 * b]["out"] + results[2 * b + 1]["out"] for b in range(B)]
    ).astype(np.float32)


def kernel(x, W_qkv, W_out):
    nc = _build_program()
    in_maps = make_in_maps(x, W_qkv, W_out)
    res = run_bass_kernel_spmd(nc, in_maps, core_ids=list(range(8)))
    return combine_outputs(res.results)


if __name__ == "__main__":
    # smoke test against a local numpy reference
    rng = np.random.default_rng(0)
    x = rng.standard_normal((B, S, D), dtype=np.float32)
    W_qkv = (rng.standard_normal((D, 3 * D)) * 0.02).astype(np.float32)
    W_out = (rng.standard_normal((D, D)) * 0.02).astype(np.float32)
    out = kernel(x, W_qkv, W_out)
    print("out", out.shape, out.dtype, float(np.abs(out).mean()))


# revision 21
# speedup vs baseline: 1.0816x; 1.0816x over previous
"""Causal self-attention (B=4, S=2048, D=1024, H=16, hd=64) on 8 TRN2 NeuronCores.

Sharding: batch 4-way x head-group 2-way. Core c = 2*b + g handles batch b and
heads [8g, 8g+8). Each core computes the QKV projection for its heads, causal
flash-style attention, and a partial output projection; the host sums the two
head-group partials per batch.

Per-core kernel layout choices:
  - q^T / k^T are produced in [hd, S] layout (head-dim on partitions) directly
    from the projection, V in [S, hd] layout via a second projection pass with
    x^T tiles as the stationary operand.
  - Attention computes S^T = K.Q^T per (128-kv-chunk x 512-q-superblock), so
    the exp() activation's PSUM->SBUF pass lands P^T directly in the layout the
    P^T.V matmul wants. A ones-column appended to V yields the softmax
    denominators from the same matmul (row 64 of the accumulator).
  - No running-max subtraction: scores are bounded (|s|/8 < ~30) so exp stays
    finite in fp32; masked positions get -1e10 before the 0.125 scale.
"""

import sys

for _p in ("/opt/trn_rl_repo",):
    if _p not in sys.path:
        sys.path.insert(0, _p)

from contextlib import ExitStack

import numpy as np

import concourse.bass as bass
import concourse.mybir as mybir
import concourse.tile as tile
from concourse import bacc
from concourse.bass_utils import run_bass_kernel_spmd

F32 = mybir.dt.float32
BF16 = mybir.dt.bfloat16
P = 128
B, S, D = 4, 2048, 1024
HD = 64          # head dim
NH = 8           # heads per core
KO = D // P      # 8 contraction chunks for the projections
QSB = 512        # q superblock (matmul free dim)
N_SB = S // QSB  # 4
N_SC = S // P    # 16 kv chunks
PSTRIPE = 512    # s-stripe for the projection phase
NEG = -1.0e10
SCALE = 0.125    # 1/sqrt(64)


def _attention_kernel(tc, out, xT, w_qk, w_v, w_out):
    nc = tc.nc
    with ExitStack() as ctx:
        const_pool = ctx.enter_context(tc.tile_pool(name="const", bufs=1))
        qkT_pool = ctx.enter_context(tc.tile_pool(name="qkT", bufs=1))
        v_pool = ctx.enter_context(tc.tile_pool(name="vsb", bufs=1))
        wqk_pool = ctx.enter_context(tc.tile_pool(name="wqk", bufs=1))
        wv_pool = ctx.enter_context(tc.tile_pool(name="wv", bufs=1))
        wout_pool = ctx.enter_context(tc.tile_pool(name="wout", bufs=1))
        xt_pool = ctx.enter_context(tc.tile_pool(name="xt", bufs=2))
        pt_pool = ctx.enter_context(tc.tile_pool(name="pt", bufs=4))
        y_pool = ctx.enter_context(tc.tile_pool(name="ysb", bufs=2))
        r_pool = ctx.enter_context(tc.tile_pool(name="recip", bufs=4))
        o_pool = ctx.enter_context(tc.tile_pool(name="osb", bufs=2))
        # one shared PSUM pool of 2-bank tiles (plus the two y accumulators):
        # 3*2 + 2*1 = 8 banks. Projection, attention scores, and the out-proj
        # accumulators all rotate through ps_s2 so projection stripes can
        # execute concurrently with attention on earlier stripes.
        ps_s2 = ctx.enter_context(tc.tile_pool(name="ps_s2", bufs=3, space="PSUM"))
        ps_y = ctx.enter_context(tc.tile_pool(name="ps_y", bufs=2, space="PSUM"))

        # 128x128 triangle for the diagonal block (transposed layout):
        # tri[i, j] = 1 if j >= i else 0
        tri = const_pool.tile([P, P], BF16, tag="tri")
        nc.gpsimd.memset(tri[:], 1.0)
        nc.gpsimd.affine_select(
            out=tri[:],
            in_=tri[:],
            compare_op=mybir.AluOpType.is_ge,
            fill=0.0,
            base=0,
            channel_multiplier=-1,
            pattern=[[1, P]],
        )

        # q^T/k^T store: row-chunk rc<4 holds q rows, rc>=4 holds k rows.
        # Head h lives at partitions 64*(h%2)..+64 of row-chunk h//2 (+4 for k).
        qkT = qkT_pool.tile([P, 8, S], BF16)
        # V store: [s-partition, kv-chunk, head, hd+1]; last col is ones for the
        # softmax denominator.
        v_sb = v_pool.tile([P, N_SC, NH, HD + 1], BF16)
        nc.gpsimd.memset(v_sb[:, :, :, HD], 1.0)

        # stripe-0 x chunks interleave with the weight chunks so the first
        # projection matmul starts after ~2 chunks instead of the full 5 MB
        wqk_sb = wqk_pool.tile([P, KO, 2 * 512], BF16)
        xt0 = xt_pool.tile([P, KO, PSTRIPE], BF16, tag="xt", name="xt_first")
        for ko in range(KO):
            nc.sync.dma_start(
                wqk_sb[:, ko, :],
                w_qk[ko * P:(ko + 1) * P, :],
            )
            nc.sync.dma_start(xt0[:, ko, :], xT[ko * P:(ko + 1) * P, 0:PSTRIPE])
        wv_sb = wv_pool.tile([P, KO, 512], BF16)
        nc.sync.dma_start(wv_sb[:], w_v.rearrange("(ko ki) n -> ki ko n", ki=P))
        wout_sb = wout_pool.tile([P, 4, D], BF16)
        nc.sync.dma_start(wout_sb[:], w_out.rearrange("(co ci) n -> ci co n", ci=P))

        def proj_stripe(st, pieces=None, xt_pre=None):
            if xt_pre is not None:
                xt = xt_pre
            else:
                xt = xt_pool.tile([P, KO, PSTRIPE], BF16, tag="xt", name=f"xt{st}")
                for ko in range(KO):
                    nc.sync.dma_start(
                        xt[:, ko, :],
                        xT[ko * P:(ko + 1) * P, st * PSTRIPE:(st + 1) * PSTRIPE],
                    )
            # q^T/k^T rows, two row-chunks per 2-bank psum tile
            def qk_piece(rcp):
                ps = ps_s2.tile([P, 2, PSTRIPE], F32, tag="ps_s2", name=f"pqk{rcp}")
                for half in range(2):
                    rc = 2 * rcp + half
                    for ko in range(KO):
                        nc.tensor.matmul(
                            ps[:, half, :],
                            lhsT=wqk_sb[:, ko, rc * P:(rc + 1) * P],
                            rhs=xt[:, ko, :],
                            start=(ko == 0),
                            stop=(ko == KO - 1),
                        )
                nc.vector.tensor_copy(
                    qkT[:, 2 * rcp:2 * rcp + 2, st * PSTRIPE:(st + 1) * PSTRIPE],
                    ps[:],
                )
            # V rows, two s-chunks per psum tile
            def v_piece(subp):
                ps = ps_s2.tile([P, 2, NH * HD], F32, tag="ps_s2", name=f"pv{subp}")
                for half in range(2):
                    sub = 2 * subp + half
                    for ko in range(KO):
                        nc.tensor.matmul(
                            ps[:, half, :],
                            lhsT=xt[:, ko, sub * P:(sub + 1) * P],
                            rhs=wv_sb[:, ko, :],
                            start=(ko == 0),
                            stop=(ko == KO - 1),
                        )
                sc0 = st * (PSTRIPE // P) + 2 * subp
                nc.vector.tensor_copy(
                    v_sb[:, sc0:sc0 + 2, :, 0:HD],
                    ps.rearrange("p t (h e) -> p t h e", h=NH),
                )

            todo = [lambda r=r: qk_piece(r) for r in range(4)]
            todo += [lambda s_=s_: v_piece(s_) for s_ in range(PSTRIPE // P // 2)]
            if pieces is None:
                for fn in todo:
                    fn()
            else:
                pieces.extend(todo)

        def attn_sb(sb, pieces=()):
            pieces = list(pieces)
            ySb = y_pool.tile([P, 4, QSB], BF16, tag="ysb", name=f"ysb{sb}")
            # head pairs: h0/h1 share a row-chunk at base partitions 0/64, so
            # their QK matmuls land on disjoint PE row-groups (issued
            # adjacently for array-level concurrency) and the interleave keeps
            # the PE fed across the exp() dependency.
            for hp in range(NH // 2):
                heads = (2 * hp, 2 * hp + 1)
                rc_k = 4 + hp
                y_pss = [
                    ps_y.tile([P, QSB], F32, tag="ps_y", name=f"yps{i}")
                    for i in range(2)
                ]
                nch = 4 * (sb + 1)
                for g in range(nch // 2):
                    c0 = 2 * g
                    # causal trim: chunk c only attends q >= c*128, i.e.
                    # columns qoff.. of this superblock; the diagonal 128x128
                    # block gets a triangular mask on P^T instead of a full
                    # additive mask pass.
                    qoffs = [P * max(0, c0 + i - 4 * sb) for i in range(2)]
                    s2s = [
                        ps_s2.tile([P, 2, QSB], F32, tag="ps_s2", name=f"s2_{i}")
                        for i in range(2)
                    ]
                    for i in range(2):
                        qo = qoffs[i]
                        for (h, s2) in zip(heads, s2s):
                            bp = (h % 2) * HD
                            c = c0 + i
                            nc.tensor.matmul(
                                s2[:, i, qo:],
                                lhsT=qkT[bp:bp + HD, rc_k, c * P:(c + 1) * P],
                                rhs=qkT[bp:bp + HD, hp, sb * QSB + qo:(sb + 1) * QSB],
                                start=True,
                                stop=True,
                            )
                    for (h, s2, y_ps) in zip(heads, s2s, y_pss):
                        bp = (h % 2) * HD
                        pt = pt_pool.tile([P, 2, QSB], BF16, tag="pt")
                        ptb = pt[:]
                        if qoffs[0] == qoffs[1]:
                            nc.scalar.activation(
                                pt[:, :, qoffs[0]:], s2[:, :, qoffs[0]:],
                                mybir.ActivationFunctionType.Exp,
                                scale=SCALE,
                            )
                        else:
                            for i in range(2):
                                nc.scalar.activation(
                                    pt[:, i, qoffs[i]:], s2[:, i, qoffs[i]:],
                                    mybir.ActivationFunctionType.Exp,
                                    scale=SCALE,
                                )
                        for i in range(2):
                            c = c0 + i
                            qo = qoffs[i]
                            if c >= 4 * sb:
                                # triangle at the causal diagonal block
                                nc.vector.tensor_tensor(
                                    ptb[:, i, qo:qo + P],
                                    ptb[:, i, qo:qo + P],
                                    tri[:],
                                    mybir.AluOpType.mult,
                                )
                            nc.tensor.matmul(
                                y_ps[0:HD + 1, qo:],
                                lhsT=v_sb[:, c, h, :],
                                rhs=ptb[:, i, qo:],
                                start=(c == 0),
                                stop=(c == nch - 1),
                            )
                if pieces:
                    pieces.pop(0)()
                    if hp == NH // 2 - 1 and pieces:
                        # flush any remainder before the out-projection
                        while pieces:
                            pieces.pop(0)()
                for h, y_ps in zip(heads, y_pss):
                    bp = (h % 2) * HD
                    # one copy releases the PSUM accumulator immediately (the
                    # next head-pair's AVs need the bank); the normalize chain
                    # then runs off the SBUF copy.
                    yc = r_pool.tile([HD + 1, QSB], F32, tag="yc")
                    nc.vector.tensor_copy(yc[:], y_ps[0:HD + 1, :])
                    # stage sums at partition 0: reciprocal_approx_fast
                    # (custom DVE op) reads garbage from nonzero base
                    # partitions on HW
                    ssum = r_pool.tile([1, QSB], F32, tag="ssum")
                    nc.vector.tensor_copy(ssum[:], yc[HD:HD + 1, :])
                    r = r_pool.tile([1, QSB], F32, tag="r")
                    nc.vector.reciprocal_approx_fast(r[:], ssum[:])
                    rbc = r_pool.tile([HD, QSB], F32, tag="rbc")
                    nc.gpsimd.partition_broadcast(rbc[:], r[:])
                    nc.vector.tensor_tensor(
                        ySb[bp:bp + HD, hp, :], yc[0:HD, :], rbc[:],
                        mybir.AluOpType.mult,
                    )
            # output projection for this superblock's s-range
            for sub in range(QSB // P):
                o_t = o_pool.tile([P, 2, 512], F32, tag="osb")
                o_ps = ps_s2.tile([P, 2, 512], F32, tag="ps_s2", name="ops")
                for nt in range(2):
                    for cc in range(4):
                        nc.tensor.matmul(
                            o_ps[:, nt, :],
                            lhsT=ySb[:, cc, sub * P:(sub + 1) * P],
                            rhs=wout_sb[:, cc, nt * 512:(nt + 1) * 512],
                            start=(cc == 0),
                            stop=(cc == 3),
                        )
                nc.vector.tensor_copy(o_t[:], o_ps[:])
                row = (sb * (QSB // P) + sub) * P
                nc.sync.dma_start(out[row:row + P, :], o_t.rearrange("p a b -> p (a b)"))

        # dovetail: attention on superblock sb only needs projection stripes
        # <= sb, so stripe sb+1's pieces are interleaved between attention
        # head-pairs of superblock sb (keeps ACT fed with exps while the PE
        # chews projection matmuls).
        proj_stripe(0, xt_pre=xt0)
        for sb in range(N_SB):
            pieces = []
            if sb + 1 < N_SB:
                proj_stripe(sb + 1, pieces)
            attn_sb(sb, pieces)


_NC_CACHE = None


def _build_program():
    global _NC_CACHE
    if _NC_CACHE is not None:
        return _NC_CACHE
    nc = bacc.Bacc("TRN2", target_bir_lowering=False, debug=False)
    xT = nc.dram_tensor("xT", [D, S], BF16, kind="ExternalInput").ap()
    w_qk = nc.dram_tensor("w_qk", [D, 1024], BF16, kind="ExternalInput").ap()
    w_v = nc.dram_tensor("w_v", [D, 512], BF16, kind="ExternalInput").ap()
    w_out = nc.dram_tensor("w_out", [512, D], BF16, kind="ExternalInput").ap()
    out = nc.dram_tensor("out", [S, D], F32, kind="ExternalOutput").ap()
    with tile.TileContext(nc) as tc:
        _attention_kernel(tc, out, xT, w_qk, w_v, w_out)
    nc.compile()
    _NC_CACHE = nc
    return nc


def make_in_maps(x, W_qkv, W_out):
    import ml_dtypes

    bf16 = ml_dtypes.bfloat16
    x = np.ascontiguousarray(np.asarray(x, dtype=np.float32))
    W_qkv = np.asarray(W_qkv, dtype=np.float32)
    W_out = np.asarray(W_out, dtype=np.float32)
    in_maps = []
    for c in range(8):
        b, g = divmod(c, 2)
        lo = 512 * g
        cols = np.arange(lo, lo + 512)
        in_maps.append({
            "xT": np.ascontiguousarray(x[b].T).astype(bf16),
            "w_qk": np.ascontiguousarray(
                np.concatenate([W_qkv[:, cols], W_qkv[:, D + cols]], axis=1)
            ).astype(bf16),
            "w_v": np.ascontiguousarray(W_qkv[:, 2 * D + cols]).astype(bf16),
            "w_out": np.ascontiguousarray(W_out[cols, :]).astype(bf16),
        })
    return in_maps


def combine_outputs(results):
    # results: list of 8 dicts with "out" [S, D]; core c = 2*b + g
    return np.stack(
        [results[2 * b]["out"] + results[2 * b + 1]["out"] for b in range(B)]
    ).astype(np.float32)


def kernel(x, W_qkv, W_out):
    nc = _build_program()
    in_maps = make_in_maps(x, W_qkv, W_out)
    res = run_bass_kernel_spmd(nc, in_maps, core_ids=list(range(8)))
    return combine_outputs(res.results)


if __name__ == "__main__":
    # smoke test against a local numpy reference
    rng = np.random.default_rng(0)
    x = rng.standard_normal((B, S, D), dtype=np.float32)
    W_qkv = (rng.standard_normal((D, 3 * D)) * 0.02).astype(np.float32)
    W_out = (rng.standard_normal((D, D)) * 0.02).astype(np.float32)
    out = kernel(x, W_qkv, W_out)
    print("out", out.shape, out.dtype, float(np.abs(out).mean()))
